# revision 41
# baseline (speedup 1.0000x reference)
"""Causal self-attention (B=4, S=2048, E=1024, H=16) on 8 TRN2 NeuronCores.

Sharding: core = (batch b, head-group g): b = core // 2, g = core % 2.

Schedule: single priority-ordered input DMA stream on the sync HWDGE ring;
attention blocks woven with QKV/proj GEMM fill so the PE never stalls on
the Scalar engine's exp.
"""

import sys

for _p in ("/opt/trn_rl_repo", "/root/.axon_site/_ro/trn_rl_repo"):
    if _p not in sys.path:
        sys.path.append(_p)

from contextlib import ExitStack

import numpy as np
import ml_dtypes

import concourse.bass as bass
import concourse.tile as tile
import concourse.mybir as mybir
from concourse import bacc
from concourse.bass_utils import run_bass_kernel_spmd

BF16 = mybir.dt.bfloat16
F32 = mybir.dt.float32
NP_BF16 = ml_dtypes.bfloat16

B, S, E, H = 4, 2048, 1024, 16
D = E // H            # 64
HL = H // 2           # 8 heads per core
JC = HL * D           # 512 local head-concat width
P = 128
NKT = S // P          # 16 key tiles
NQT = S // 512        # 4 query tiles of 512
EKT = E // P          # 8 contraction tiles for QKV projections
CT = JC // P          # 4 contraction tiles for the output projection
SCALE = 1.0 / np.sqrt(np.float32(D))  # 0.125


def build_program(apply_key_mask: bool):
    nc = bacc.Bacc("TRN2", target_bir_lowering=False, debug=False, num_devices=8)

    xT = nc.dram_tensor("xT", [E, S], BF16, kind="ExternalInput").ap()
    wqT = nc.dram_tensor("wqT", [E, JC], BF16, kind="ExternalInput").ap()
    wkT = nc.dram_tensor("wkT", [E, JC], BF16, kind="ExternalInput").ap()
    wvT = nc.dram_tensor("wvT", [E, JC], BF16, kind="ExternalInput").ap()
    wpT = nc.dram_tensor("wpT", [JC, E], BF16, kind="ExternalInput").ap()
    cmask = nc.dram_tensor("cmask", [P, P], BF16, kind="ExternalInput").ap()
    ident = nc.dram_tensor("ident", [D, D], BF16, kind="ExternalInput").ap()
    if apply_key_mask:
        kmaskT = nc.dram_tensor("kmaskT", [P, NKT], F32, kind="ExternalInput").ap()
    outp = nc.dram_tensor("outp", [E, S], BF16, kind="ExternalOutput").ap()

    xT_r = xT.rearrange("(kt p) (q c) -> p kt q c", p=P, c=512)
    wq_r = wqT.rearrange("(kt p) j -> p kt j", p=P)
    wk_r = wkT.rearrange("(kt p) j -> p kt j", p=P)
    wv_r = wvT.rearrange("(kt p) j -> p kt j", p=P)
    wp_r = wpT.rearrange("(ct p) e -> p ct e", p=P)
    out_r = outp.rearrange("(et p) s -> p et s", p=P)

    with tile.TileContext(nc) as tc:
        with ExitStack() as ctx:
            per = ctx.enter_context(tc.tile_pool(name="per", bufs=1))
            sc_ps = ctx.enter_context(
                tc.tile_pool(name="sc_ps", bufs=2, space="PSUM")
            )
            pv_ps = ctx.enter_context(
                tc.tile_pool(name="pv_ps", bufs=2, space="PSUM")
            )
            fill_ps = ctx.enter_context(
                tc.tile_pool(name="fill_ps", bufs=2, space="PSUM")
            )
            esb = ctx.enter_context(tc.tile_pool(name="esb", bufs=6))
            nrm = ctx.enter_context(tc.tile_pool(name="nrm", bufs=3))
            posb = ctx.enter_context(tc.tile_pool(name="posb", bufs=3))

            # SBUF tiles
            xT_sb = per.tile([P, EKT, S], BF16, tag="xT")
            wq_sb = per.tile([P, EKT, JC], BF16, tag="wq")
            wk_sb = per.tile([P, EKT, JC], BF16, tag="wk")
            wv_sb = per.tile([P, EKT, JC], BF16, tag="wv")
            wp_sb = per.tile([P, CT, E], BF16, tag="wp")
            cm_sb = per.tile([P, P], BF16, tag="cm")
            if apply_key_mask:
                km_sb = per.tile([P, NKT], F32, tag="km")

            qT_sb = per.tile([P, CT, S], BF16, tag="qT")
            kT_sb = per.tile([P, CT, S], BF16, tag="kT")
            vaug_sb = per.tile([P, NKT, HL, D + 1], BF16, tag="vaug")
            o_t = [
                [per.tile([P, 512], BF16, tag=f"o{a}_{st}", name=f"o{a}_{st}")
                 for st in range(NQT)]
                for a in range(CT)
            ]

            # ---- input DMA: one priority-ordered stream on the sync HWDGE
            # ring.  A single transfer already spans all 16 SDMA engines, so
            # ordering IS the prefetch policy.  Keep pieces >= 512KB: the
            # per-transfer fixed cost (~1-2us completion receipt) dominates
            # smaller ones.
            def ld_x(q, kts):
                nc.sync.dma_start(
                    xT_sb[:, kts, q * 512:(q + 1) * 512], xT_r[:, kts, q]
                )

            # Two parallel HWDGE FIFO rings (sync + scalar): per-transfer
            # completion receipts serialize within a ring, so splitting the
            # stream across both nearly doubles effective input bandwidth.
            # The scalar ring's triggers all issue before the first exp.
            # tiny constants lead each ring to absorb the per-ring
            # cold-transfer setup penalty, so the 1MB weights run warm
            ident_sb = per.tile([D, D], BF16, tag="ident")
            nc.sync.dma_start(cm_sb[:], cmask[:])
            nc.scalar.dma_start(ident_sb[:], ident[:])
            nc.sync.dma_start(wq_sb[:], wq_r[:])
            nc.scalar.dma_start(
                xT_sb[:, 0:4, 0:512], xT_r[:, 0:4, 0]
            )
            nc.scalar.dma_start(wk_sb[:], wk_r[:])
            ld_x(0, slice(4, 8))
            nc.sync.dma_start(wv_sb[:], wv_r[:])
            if apply_key_mask:
                nc.scalar.dma_start(km_sb[:], kmaskT[:])
            nc.scalar.dma_start(
                xT_sb[:, 0:8, 1024:1536], xT_r[:, 0:8, 2]
            )
            ld_x(1, slice(0, 8))
            nc.sync.dma_start(wp_sb[:], wp_r[:])
            ld_x(3, slice(0, 8))

            # PE warm-up while the first DMAs land
            dum_a = per.tile([P, P], BF16, tag="dum_a")
            dum_b = per.tile([P, 512], BF16, tag="dum_b")
            nc.vector.memset(dum_a[:], 0.0)
            nc.vector.memset(dum_b[:], 0.0)
            dps = fill_ps.tile([P, 512], F32, tag="ps")
            for i in range(14):
                nc.tensor.matmul(
                    dps[:], dum_a[:], dum_b[:],
                    start=(i == 0), stop=(i == 13),
                )

            # v-augmentation ones row (softmax denominator via matmul)
            nc.vector.memset(vaug_sb[:, :, :, D], 1.0)
            # all-ones column for the PE-side partition broadcast in the
            # final normalize
            ones_sb = per.tile([1, D], F32, tag="ones")
            nc.vector.memset(ones_sb[:], 1.0)

            # preload the exp table set on ScalarE
            warm = nrm.tile([1, 16], F32, tag="warm")
            nc.vector.memset(warm[:], 0.0)
            warm2 = nrm.tile([1, 16], F32, tag="warm2")
            nc.scalar.activation(
                warm2[:], warm[:], mybir.ActivationFunctionType.Exp
            )

            # ---- GEMM fill units -------------------------------------
            def emit_q(jt, dst, st):
                """One QK projection group: 8 matmuls + cast (~1.7us PE)."""
                w_sb, out_sb = (wq_sb, qT_sb) if dst == 0 else (wk_sb, kT_sb)
                ps = fill_ps.tile([P, 512], F32, tag="ps")
                for kt in range(EKT):
                    nc.tensor.matmul(
                        ps[:],
                        w_sb[:, kt, jt * P:(jt + 1) * P],
                        xT_sb[:, kt, st * 512:(st + 1) * 512],
                        start=(kt == 0),
                        stop=(kt == EKT - 1),
                        skip_group_check=True,
                    )
                nc.vector.tensor_copy(
                    out_sb[:, jt, st * 512:(st + 1) * 512], ps[:]
                )

            def emit_v(st):
                """One V projection group for a 128-query tile (~1.7us PE)."""
                ps = fill_ps.tile([P, 512], F32, tag="ps")
                for kt in range(EKT):
                    nc.tensor.matmul(
                        ps[:],
                        xT_sb[:, kt, st * P:(st + 1) * P],
                        wv_sb[:, kt, :],
                        start=(kt == 0),
                        stop=(kt == EKT - 1),
                        skip_group_check=True,
                    )
                nc.vector.tensor_copy(
                    vaug_sb[:, st, :, 0:D],
                    ps[:].rearrange("p (h d) -> p h d", d=D),
                )
                if apply_key_mask:
                    nc.vector.tensor_scalar_mul(
                        vaug_sb[:, st], vaug_sb[:, st], km_sb[:, st:st + 1]
                    )

            def emit_proj_pair(st, ep, fine_store=False):
                """Two proj et-groups + one batched store (~1.7us PE)."""
                po = posb.tile([P, 2, 512], BF16, tag="po")
                for i in range(2):
                    et = 2 * ep + i
                    ps = fill_ps.tile([P, 512], F32, tag="ps")
                    for ct in range(CT):
                        nc.tensor.matmul(
                            ps[:],
                            wp_sb[:, ct, et * P:(et + 1) * P],
                            o_t[ct][st][:, :],
                            start=(ct == 0),
                            stop=(ct == CT - 1),
                            skip_group_check=True,
                        )
                    nc.vector.tensor_copy(po[:, i], ps[:])
                    if fine_store:
                        nc.sync.dma_start(
                            out_r[:, et, st * 512:(st + 1) * 512], po[:, i]
                        )
                if not fine_store:
                    nc.sync.dma_start(
                        out_r[:, 2 * ep:2 * ep + 2, st * 512:(st + 1) * 512],
                        po[:],
                    )

            # ---- attention stretch with woven GEMM fill ---------------
            def emit_attn(a, qt, fills=(), boot_vs=False, tail=False,
                          fill_caps=()):
                qs0 = qt * 512
                nkt = 4 * qt + 4
                pv = pv_ps.tile([P, 512], F32, tag="pv")
                pv2 = pv_ps.tile([P, 512], F32, tag="pv")
                fill_q = list(fills)
                nf = len(fill_q)
                # even spacing, clamped by per-fill caps (a V fill for key
                # tile j must fire before block j reads it)
                pos = [(i * nkt) // nf if nf else 0 for i in range(nf)]
                for i, c in enumerate(fill_caps):
                    if c is not None:
                        pos[i] = min(pos[i], c)
                done_f = 0

                def emit_pv(kt):
                    r = kt - 4 * qt
                    c0 = 128 * r if r > 0 else 0
                    nc.tensor.matmul(
                        pv[0:D + 1, c0:512],
                        vaug_sb[:, kt, 2 * a, :],
                        e_t[kt][:, c0:512],
                        start=(kt == 0),
                        stop=(kt == nkt - 1),
                        skip_group_check=True,
                    )
                    nc.tensor.matmul(
                        pv2[0:D + 1, c0:512],
                        vaug_sb[:, kt, 2 * a + 1, :],
                        e_t[kt][:, 512 + c0:1024],
                        start=(kt == 0),
                        stop=(kt == nkt - 1),
                        skip_group_check=True,
                    )

                # software-pipelined by one block: sc/exp of block kt issue
                # before pv of block kt-1, so the pv never waits on the exp
                e_t = {}
                for kt in range(nkt):
                    r = kt - 4 * qt
                    c0 = 128 * r if r > 0 else 0
                    ks = slice(kt * P, (kt + 1) * P)
                    qs = slice(qs0 + c0, qs0 + 512)
                    sc = sc_ps.tile([P, 1024], F32, tag="sc")
                    nc.tensor.matmul(
                        sc[:, c0:512],
                        kT_sb[0:D, a, ks],
                        qT_sb[0:D, a, qs],
                        start=True,
                        stop=True,
                    )
                    nc.tensor.matmul(
                        sc[:, 512 + c0:1024],
                        kT_sb[D:2 * D, a, ks],
                        qT_sb[D:2 * D, a, qs],
                        start=True,
                        stop=True,
                    )
                    e = esb.tile([P, 1024], BF16, tag="e")
                    e_t[kt] = e
                    e2 = e[:].rearrange("p (two c) -> p two c", two=2)
                    sc2 = sc[:].rearrange("p (two c) -> p two c", two=2)
                    nc.scalar.activation(
                        e2[:, :, c0:512], sc2[:, :, c0:512],
                        mybir.ActivationFunctionType.Exp,
                        scale=float(SCALE),
                    )
                    if r >= 0:
                        nc.vector.tensor_mul(
                            e2[:, :, c0:c0 + 128],
                            e2[:, :, c0:c0 + 128],
                            cm_sb[:, None, 0:128].to_broadcast((P, 2, 128)),
                        )
                    # evenly spaced fill injection keeps GEMM work between
                    # the sc and pv of consecutive blocks so the pv (which
                    # needs this block's exp) never reaches the PE early
                    while done_f < nf and kt >= pos[done_f]:
                        fill_q[done_f]()
                        done_f += 1
                    if boot_vs:
                        # bootstrap: the V group for this key tile must be
                        # emitted before the pv matmul that consumes it (pv
                        # of block kt fires next iteration); after the fills
                        # so the wv arrival hides behind fill GEMMs
                        emit_v(kt)
                    if kt >= 1:
                        emit_pv(kt - 1)
                emit_pv(nkt - 1)
                for f in fill_q[done_f:]:
                    f()
                # normalize: o = pv[0:64] * (1 / pv[64]).  h1 first (its
                # GpSimd partition-shifted mul is the longer path); the
                # denominator row is copied straight from PSUM (shifted DVE
                # read works), and the odd head's mul runs on GpSimd which
                # CAN write partitions 64..127 from inputs at 0..63.
                if tail:
                    # last chain gates the final projections: broadcasts on
                    # the (now idle) PE, and the h1 partition shift via a
                    # small identity matmul + cast instead of a DMA (saves
                    # the ~2us HBM-receipt-class completion latency)
                    bcp = sc_ps.tile([P, 1024], F32, tag="sc")
                    for i, pvt in enumerate((pv2, pv)):
                        ud = nrm.tile([1, 512], F32, tag="ud")
                        nc.vector.tensor_copy(ud[:], pvt[D:D + 1, :])
                        rec = nrm.tile([1, 512], F32, tag="rec")
                        nc.vector.reciprocal_approx_fast(rec[:], ud[:])
                        nc.tensor.matmul(
                            bcp[0:D, 512 * i:512 * (i + 1)], ones_sb[:],
                            rec[:], start=True, stop=True,
                            skip_group_check=True,
                        )
                    u2 = nrm.tile([D, 512], F32, tag="u")
                    nc.vector.tensor_copy(u2[:], pv2[0:D, :])
                    tmp = nrm.tile([D, 512], BF16, tag="tmp")
                    nc.vector.tensor_mul(tmp[:], u2[:], bcp[0:D, 0:512])
                    nc.tensor.matmul(
                        bcp[D:2 * D, 512:1024], ident_sb[:], tmp[:],
                        start=True, stop=True, skip_group_check=True,
                    )
                    nc.vector.tensor_copy(
                        o_t[a][qt][D:2 * D, :], bcp[D:2 * D, 512:1024]
                    )
                    u0 = nrm.tile([D, 512], F32, tag="u")
                    nc.vector.tensor_copy(u0[:], pv[0:D, :])
                    nc.vector.tensor_mul(
                        o_t[a][qt][0:D, :], u0[:], bcp[0:D, 512:1024]
                    )
                    return
                for h_par, pvt in ((1, pv2), (0, pv)):
                    ud = nrm.tile([1, 512], F32, tag="ud")
                    nc.vector.tensor_copy(ud[:], pvt[D:D + 1, :])
                    rec = nrm.tile([1, 512], F32, tag="rec")
                    nc.vector.reciprocal_approx_fast(rec[:], ud[:])
                    u = nrm.tile([D, 512], F32, tag="u")
                    nc.vector.tensor_copy(u[:], pvt[0:D, :])
                    bc = nrm.tile([D, 512], F32, tag="bc")
                    nc.gpsimd.partition_broadcast(bc[:], rec[:])
                    if h_par == 0:
                        nc.vector.tensor_mul(
                            o_t[a][qt][0:D, :], u[:], bc[:]
                        )
                    else:
                        tmp = nrm.tile([D, 512], BF16, tag="tmp")
                        nc.vector.tensor_mul(tmp[:], u[:], bc[:])
                        # shift to partitions 64..127 (DVE cannot)
                        nc.sync.dma_start(o_t[a][qt][D:2 * D, :], tmp[:])

            # ---- schedule ---------------------------------------------
            Q = lambda jt, dst, st: (lambda: emit_q(jt, dst, st))
            V = lambda st: (lambda: emit_v(st))
            Pp = lambda st, ep, fine=False: (
                lambda: emit_proj_pair(st, ep, fine_store=fine)
            )

            # Each stretch's fills lead with the NEXT stretch's QK groups so
            # their casts are done well before that stretch's first sc.
            # Closing phase uses the smallest (qt=0) stretches so the end of
            # the kernel is never exp-limited.
            emit_q(0, 0, 0)
            emit_q(0, 1, 0)
            emit_attn(0, 0, boot_vs=True,
                      fills=[Q(1, 0, 0), Q(1, 1, 0)])        # S1 + V0..3
            emit_attn(1, 0, fills=[Q(0, 0, 1), Q(0, 1, 1),
                                   V(4)])                    # S2
            emit_attn(0, 1, fills=[Q(1, 0, 1), Q(1, 1, 1),
                                   V(5), V(6), V(7)],
                      fill_caps=(None, None, 4, 5, 6))       # S3
            emit_attn(1, 1, fills=[Q(2, 0, 0), Q(2, 1, 0),
                                   V(8)])                    # S4
            emit_attn(2, 0, fills=[Q(2, 0, 1), Q(2, 1, 1)])  # S5
            emit_attn(2, 1, fills=[Q(3, 0, 0), Q(3, 1, 0),
                                   V(9), V(10)])             # S6
            emit_attn(3, 0, fills=[Q(3, 0, 1), Q(3, 1, 1)])  # S7
            emit_attn(3, 1, fills=[Q(0, 1, 2), Q(0, 1, 3),
                                   Q(0, 0, 3), V(11), V(12)])  # S8
            emit_attn(0, 3, fills=[Q(1, 1, 2), Q(1, 1, 3),
                                   Q(1, 0, 3), V(13), V(14),
                                   V(15)],
                      fill_caps=(None, None, None,
                                 12, 13, 14))                # S9
            emit_attn(1, 3, fills=[Q(2, 1, 2), Q(2, 1, 3),
                                   Q(2, 0, 3), Pp(0, 0)])    # S10
            emit_attn(2, 3, fills=[Q(3, 1, 2), Q(3, 1, 3),
                                   Q(3, 0, 3), Pp(0, 1)])    # S11
            emit_attn(3, 3, fills=[Q(0, 0, 2), Pp(0, 2),
                                   Pp(0, 3), Pp(1, 0)])      # S12
            emit_attn(0, 2, fills=[Q(1, 0, 2), Pp(1, 1),
                                   Pp(1, 2)])                # S13
            emit_attn(1, 2, fills=[Q(2, 0, 2), Pp(1, 3),
                                   Pp(3, 0)])                # S14
            emit_attn(2, 2, fills=[Q(3, 0, 2), Pp(3, 1)])   # S15
            emit_attn(3, 2, fills=[Pp(3, 2), Pp(3, 3)],
                      tail=True)                             # S16
            # tail: final projections for st=2
            emit_proj_pair(2, 0)
            emit_proj_pair(2, 1)
            emit_proj_pair(2, 2)
            emit_proj_pair(2, 3, fine_store=True)

    nc.compile()
    return nc


def _causal_mask128() -> np.ndarray:
    p = np.arange(P)[:, None]
    c = np.arange(P)[None, :]
    return (c >= p).astype(np.float32).astype(NP_BF16)


def kernel(input, attention_mask, Wq, Wk, Wv, Wp, _profile=False):
    input = np.asarray(input, dtype=np.float32)
    attention_mask = np.asarray(attention_mask)
    Wq, Wk, Wv, Wp = (np.asarray(w, dtype=np.float32) for w in (Wq, Wk, Wv, Wp))

    mask_all = bool(attention_mask.all())
    nc = build_program(apply_key_mask=not mask_all)

    cm = _causal_mask128()
    in_maps = []
    for core in range(8):
        b, g = core // 2, core % 2
        rows = slice(g * JC, (g + 1) * JC)
        m = {
            "xT": np.ascontiguousarray(input[b].T).astype(NP_BF16),
            "wqT": np.ascontiguousarray(Wq[rows].T).astype(NP_BF16),
            "wkT": np.ascontiguousarray(Wk[rows].T).astype(NP_BF16),
            "wvT": np.ascontiguousarray(Wv[rows].T).astype(NP_BF16),
            "wpT": np.ascontiguousarray(Wp[:, rows].T).astype(NP_BF16),
            "cmask": cm,
            "ident": np.eye(D, dtype=np.float32).astype(NP_BF16),
        }
        if not mask_all:
            km = attention_mask[b].astype(np.float32)  # [S]
            m["kmaskT"] = np.ascontiguousarray(km.reshape(NKT, P).T)
        in_maps.append(m)

    res = run_bass_kernel_spmd(
        nc, in_maps, core_ids=list(range(8)), trace=_profile
    )

    out = np.empty((B, S, E), dtype=np.float32)
    for b in range(B):
        acc = (res.results[2 * b]["outp"].astype(np.float32)
               + res.results[2 * b + 1]["outp"].astype(np.float32))
        out[b] = acc.T
    if _profile:
        return out, res
    return out


# revision 42
# speedup vs baseline: 1.0096x; 1.0096x over previous
"""Causal self-attention (B=4, S=2048, E=1024, H=16) on 8 TRN2 NeuronCores.

Sharding: core = (batch b, head-group g): b = core // 2, g = core % 2.

Schedule: single priority-ordered input DMA stream on the sync HWDGE ring;
attention blocks woven with QKV/proj GEMM fill so the PE never stalls on
the Scalar engine's exp.
"""

import sys

for _p in ("/opt/trn_rl_repo", "/root/.axon_site/_ro/trn_rl_repo"):
    if _p not in sys.path:
        sys.path.append(_p)

from contextlib import ExitStack

import numpy as np
import ml_dtypes

import concourse.bass as bass
import concourse.tile as tile
import concourse.mybir as mybir
from concourse import bacc
from concourse.bass_utils import run_bass_kernel_spmd

BF16 = mybir.dt.bfloat16
F32 = mybir.dt.float32
NP_BF16 = ml_dtypes.bfloat16

B, S, E, H = 4, 2048, 1024, 16
D = E // H            # 64
HL = H // 2           # 8 heads per core
JC = HL * D           # 512 local head-concat width
P = 128
NKT = S // P          # 16 key tiles
NQT = S // 512        # 4 query tiles of 512
EKT = E // P          # 8 contraction tiles for QKV projections
CT = JC // P          # 4 contraction tiles for the output projection
SCALE = 1.0 / np.sqrt(np.float32(D))  # 0.125


def build_program(apply_key_mask: bool):
    nc = bacc.Bacc("TRN2", target_bir_lowering=False, debug=False, num_devices=8)

    xT = nc.dram_tensor("xT", [E, S], BF16, kind="ExternalInput").ap()
    wqT = nc.dram_tensor("wqT", [E, JC], BF16, kind="ExternalInput").ap()
    wkT = nc.dram_tensor("wkT", [E, JC], BF16, kind="ExternalInput").ap()
    wvT = nc.dram_tensor("wvT", [E, JC], BF16, kind="ExternalInput").ap()
    wpT = nc.dram_tensor("wpT", [JC, E], BF16, kind="ExternalInput").ap()
    cmask = nc.dram_tensor("cmask", [P, P], BF16, kind="ExternalInput").ap()
    ident = nc.dram_tensor("ident", [D, D], BF16, kind="ExternalInput").ap()
    if apply_key_mask:
        kmaskT = nc.dram_tensor("kmaskT", [P, NKT], F32, kind="ExternalInput").ap()
    outp = nc.dram_tensor("outp", [E, S], BF16, kind="ExternalOutput").ap()

    xT_r = xT.rearrange("(kt p) (q c) -> p kt q c", p=P, c=512)
    wq_r = wqT.rearrange("(kt p) j -> p kt j", p=P)
    wk_r = wkT.rearrange("(kt p) j -> p kt j", p=P)
    wv_r = wvT.rearrange("(kt p) j -> p kt j", p=P)
    wp_r = wpT.rearrange("(ct p) e -> p ct e", p=P)
    out_r = outp.rearrange("(et p) s -> p et s", p=P)

    with tile.TileContext(nc) as tc:
        with ExitStack() as ctx:
            per = ctx.enter_context(tc.tile_pool(name="per", bufs=1))
            sc_ps = ctx.enter_context(
                tc.tile_pool(name="sc_ps", bufs=2, space="PSUM")
            )
            pv_ps = ctx.enter_context(
                tc.tile_pool(name="pv_ps", bufs=2, space="PSUM")
            )
            fill_ps = ctx.enter_context(
                tc.tile_pool(name="fill_ps", bufs=2, space="PSUM")
            )
            esb = ctx.enter_context(tc.tile_pool(name="esb", bufs=6))
            nrm = ctx.enter_context(tc.tile_pool(name="nrm", bufs=3))
            posb = ctx.enter_context(tc.tile_pool(name="posb", bufs=3))

            # SBUF tiles
            xT_sb = per.tile([P, EKT, S], BF16, tag="xT")
            wq_sb = per.tile([P, EKT, JC], BF16, tag="wq")
            wk_sb = per.tile([P, EKT, JC], BF16, tag="wk")
            wv_sb = per.tile([P, EKT, JC], BF16, tag="wv")
            wp_sb = per.tile([P, CT, E], BF16, tag="wp")
            cm_sb = per.tile([P, P], BF16, tag="cm")
            if apply_key_mask:
                km_sb = per.tile([P, NKT], F32, tag="km")

            qT_sb = per.tile([P, CT, S], BF16, tag="qT")
            kT_sb = per.tile([P, CT, S], BF16, tag="kT")
            vaug_sb = per.tile([P, NKT, HL, D + 1], BF16, tag="vaug")
            o_t = [
                [per.tile([P, 512], BF16, tag=f"o{a}_{st}", name=f"o{a}_{st}")
                 for st in range(NQT)]
                for a in range(CT)
            ]

            # ---- input DMA: one priority-ordered stream on the sync HWDGE
            # ring.  A single transfer already spans all 16 SDMA engines, so
            # ordering IS the prefetch policy.  Keep pieces >= 512KB: the
            # per-transfer fixed cost (~1-2us completion receipt) dominates
            # smaller ones.
            def ld_x(q, kts):
                nc.sync.dma_start(
                    xT_sb[:, kts, q * 512:(q + 1) * 512], xT_r[:, kts, q]
                )

            # Two parallel HWDGE FIFO rings (sync + scalar): per-transfer
            # completion receipts serialize within a ring, so splitting the
            # stream across both nearly doubles effective input bandwidth.
            # The scalar ring's triggers all issue before the first exp.
            # tiny constants lead each ring to absorb the per-ring
            # cold-transfer setup penalty, so the 1MB weights run warm
            ident_sb = per.tile([D, D], BF16, tag="ident")
            nc.sync.dma_start(cm_sb[:], cmask[:])
            nc.scalar.dma_start(ident_sb[:], ident[:])
            nc.sync.dma_start(wq_sb[:], wq_r[:])
            nc.scalar.dma_start(
                xT_sb[:, 0:4, 0:512], xT_r[:, 0:4, 0]
            )
            nc.scalar.dma_start(wk_sb[:], wk_r[:])
            ld_x(0, slice(4, 8))
            nc.sync.dma_start(wv_sb[:], wv_r[:])
            if apply_key_mask:
                nc.scalar.dma_start(km_sb[:], kmaskT[:])
            nc.scalar.dma_start(
                xT_sb[:, 0:8, 1024:1536], xT_r[:, 0:8, 2]
            )
            ld_x(1, slice(0, 8))
            nc.sync.dma_start(wp_sb[:], wp_r[:])
            ld_x(3, slice(0, 8))

            # PE warm-up while the first DMAs land
            dum_a = per.tile([P, P], BF16, tag="dum_a")
            dum_b = per.tile([P, 512], BF16, tag="dum_b")
            nc.vector.memset(dum_a[:], 0.0)
            nc.vector.memset(dum_b[:], 0.0)
            dps = fill_ps.tile([P, 512], F32, tag="ps")
            for i in range(14):
                nc.tensor.matmul(
                    dps[:], dum_a[:], dum_b[:],
                    start=(i == 0), stop=(i == 13),
                )

            # v-augmentation ones row (softmax denominator via matmul)
            nc.vector.memset(vaug_sb[:, :, :, D], 1.0)
            # all-ones column for the PE-side partition broadcast in the
            # final normalize
            ones_sb = per.tile([1, D], F32, tag="ones")
            nc.vector.memset(ones_sb[:], 1.0)

            # preload the exp table set on ScalarE
            warm = nrm.tile([1, 16], F32, tag="warm")
            nc.vector.memset(warm[:], 0.0)
            warm2 = nrm.tile([1, 16], F32, tag="warm2")
            nc.scalar.activation(
                warm2[:], warm[:], mybir.ActivationFunctionType.Exp
            )

            # ---- GEMM fill units -------------------------------------
            def emit_q(jt, dst, st):
                """One QK projection group: 8 matmuls + cast (~1.7us PE)."""
                w_sb, out_sb = (wq_sb, qT_sb) if dst == 0 else (wk_sb, kT_sb)
                ps = fill_ps.tile([P, 512], F32, tag="ps")
                for kt in range(EKT):
                    nc.tensor.matmul(
                        ps[:],
                        w_sb[:, kt, jt * P:(jt + 1) * P],
                        xT_sb[:, kt, st * 512:(st + 1) * 512],
                        start=(kt == 0),
                        stop=(kt == EKT - 1),
                        skip_group_check=True,
                    )
                nc.vector.tensor_copy(
                    out_sb[:, jt, st * 512:(st + 1) * 512], ps[:]
                )

            def emit_v(st):
                """One V projection group for a 128-query tile (~1.7us PE)."""
                ps = fill_ps.tile([P, 512], F32, tag="ps")
                for kt in range(EKT):
                    nc.tensor.matmul(
                        ps[:],
                        xT_sb[:, kt, st * P:(st + 1) * P],
                        wv_sb[:, kt, :],
                        start=(kt == 0),
                        stop=(kt == EKT - 1),
                        skip_group_check=True,
                    )
                nc.vector.tensor_copy(
                    vaug_sb[:, st, :, 0:D],
                    ps[:].rearrange("p (h d) -> p h d", d=D),
                )
                if apply_key_mask:
                    nc.vector.tensor_scalar_mul(
                        vaug_sb[:, st], vaug_sb[:, st], km_sb[:, st:st + 1]
                    )

            def emit_proj_pair(st, ep, fine_store=False):
                """Two proj et-groups + one batched store (~1.7us PE)."""
                po = posb.tile([P, 2, 512], BF16, tag="po")
                for i in range(2):
                    et = 2 * ep + i
                    ps = fill_ps.tile([P, 512], F32, tag="ps")
                    for ct in range(CT):
                        nc.tensor.matmul(
                            ps[:],
                            wp_sb[:, ct, et * P:(et + 1) * P],
                            o_t[ct][st][:, :],
                            start=(ct == 0),
                            stop=(ct == CT - 1),
                            skip_group_check=True,
                        )
                    nc.vector.tensor_copy(po[:, i], ps[:])
                    if fine_store:
                        nc.sync.dma_start(
                            out_r[:, et, st * 512:(st + 1) * 512], po[:, i]
                        )
                if not fine_store:
                    nc.sync.dma_start(
                        out_r[:, 2 * ep:2 * ep + 2, st * 512:(st + 1) * 512],
                        po[:],
                    )

            # ---- attention stretch with woven GEMM fill ---------------
            def emit_attn(a, qt, fills=(), boot_vs=False, tail=False,
                          fill_caps=()):
                qs0 = qt * 512
                nkt = 4 * qt + 4
                pv = pv_ps.tile([P, 512], F32, tag="pv")
                pv2 = pv_ps.tile([P, 512], F32, tag="pv")
                fill_q = list(fills)
                nf = len(fill_q)
                # even spacing, clamped by per-fill caps (a V fill for key
                # tile j must fire before block j reads it)
                pos = [(i * nkt) // nf if nf else 0 for i in range(nf)]
                for i, c in enumerate(fill_caps):
                    if c is not None:
                        pos[i] = min(pos[i], c)
                done_f = 0

                def emit_pv(kt):
                    r = kt - 4 * qt
                    c0 = 128 * r if r > 0 else 0
                    nc.tensor.matmul(
                        pv[0:D + 1, c0:512],
                        vaug_sb[:, kt, 2 * a, :],
                        e_t[kt][:, c0:512],
                        start=(kt == 0),
                        stop=(kt == nkt - 1),
                        skip_group_check=True,
                    )
                    nc.tensor.matmul(
                        pv2[0:D + 1, c0:512],
                        vaug_sb[:, kt, 2 * a + 1, :],
                        e_t[kt][:, 512 + c0:1024],
                        start=(kt == 0),
                        stop=(kt == nkt - 1),
                        skip_group_check=True,
                    )

                # software-pipelined by one block: sc/exp of block kt issue
                # before pv of block kt-1, so the pv never waits on the exp
                e_t = {}
                for kt in range(nkt):
                    r = kt - 4 * qt
                    c0 = 128 * r if r > 0 else 0
                    ks = slice(kt * P, (kt + 1) * P)
                    qs = slice(qs0 + c0, qs0 + 512)
                    sc = sc_ps.tile([P, 1024], F32, tag="sc")
                    nc.tensor.matmul(
                        sc[:, c0:512],
                        kT_sb[0:D, a, ks],
                        qT_sb[0:D, a, qs],
                        start=True,
                        stop=True,
                    )
                    nc.tensor.matmul(
                        sc[:, 512 + c0:1024],
                        kT_sb[D:2 * D, a, ks],
                        qT_sb[D:2 * D, a, qs],
                        start=True,
                        stop=True,
                    )
                    e = esb.tile([P, 1024], BF16, tag="e")
                    e_t[kt] = e
                    e2 = e[:].rearrange("p (two c) -> p two c", two=2)
                    sc2 = sc[:].rearrange("p (two c) -> p two c", two=2)
                    nc.scalar.activation(
                        e2[:, :, c0:512], sc2[:, :, c0:512],
                        mybir.ActivationFunctionType.Exp,
                        scale=float(SCALE),
                    )
                    if r >= 0:
                        nc.vector.tensor_mul(
                            e2[:, :, c0:c0 + 128],
                            e2[:, :, c0:c0 + 128],
                            cm_sb[:, None, 0:128].to_broadcast((P, 2, 128)),
                        )
                    # evenly spaced fill injection keeps GEMM work between
                    # the sc and pv of consecutive blocks so the pv (which
                    # needs this block's exp) never reaches the PE early
                    while done_f < nf and kt >= pos[done_f]:
                        fill_q[done_f]()
                        done_f += 1
                    if boot_vs:
                        # bootstrap: the V group for this key tile must be
                        # emitted before the pv matmul that consumes it (pv
                        # of block kt fires next iteration); after the fills
                        # so the wv arrival hides behind fill GEMMs
                        emit_v(kt)
                    if kt >= 1:
                        emit_pv(kt - 1)
                emit_pv(nkt - 1)
                for f in fill_q[done_f:]:
                    f()
                # normalize: o = pv[0:64] * (1 / pv[64]).  h1 first (its
                # GpSimd partition-shifted mul is the longer path); the
                # denominator row is copied straight from PSUM (shifted DVE
                # read works), and the odd head's mul runs on GpSimd which
                # CAN write partitions 64..127 from inputs at 0..63.
                if tail:
                    # last chain gates the final projections: broadcasts on
                    # the (now idle) PE, and the h1 partition shift via a
                    # small identity matmul + cast instead of a DMA (saves
                    # the ~2us HBM-receipt-class completion latency)
                    bcp = sc_ps.tile([P, 1024], F32, tag="sc")
                    for i, pvt in enumerate((pv2, pv)):
                        ud = nrm.tile([1, 512], F32, tag="ud")
                        nc.vector.tensor_copy(ud[:], pvt[D:D + 1, :])
                        rec = nrm.tile([1, 512], F32, tag="rec")
                        nc.vector.reciprocal_approx_fast(rec[:], ud[:])
                        nc.tensor.matmul(
                            bcp[0:D, 512 * i:512 * (i + 1)], ones_sb[:],
                            rec[:], start=True, stop=True,
                            skip_group_check=True,
                        )
                    u2 = nrm.tile([D, 512], F32, tag="u")
                    nc.vector.tensor_copy(u2[:], pv2[0:D, :])
                    tmp = nrm.tile([D, 512], BF16, tag="tmp")
                    nc.vector.tensor_mul(tmp[:], u2[:], bcp[0:D, 0:512])
                    nc.tensor.matmul(
                        bcp[D:2 * D, 512:1024], ident_sb[:], tmp[:],
                        start=True, stop=True, skip_group_check=True,
                    )
                    nc.vector.tensor_copy(
                        o_t[a][qt][D:2 * D, :], bcp[D:2 * D, 512:1024]
                    )
                    u0 = nrm.tile([D, 512], F32, tag="u")
                    nc.vector.tensor_copy(u0[:], pv[0:D, :])
                    nc.vector.tensor_mul(
                        o_t[a][qt][0:D, :], u0[:], bcp[0:D, 512:1024]
                    )
                    return
                for h_par, pvt in ((1, pv2), (0, pv)):
                    ud = nrm.tile([1, 512], F32, tag="ud")
                    nc.vector.tensor_copy(ud[:], pvt[D:D + 1, :])
                    rec = nrm.tile([1, 512], F32, tag="rec")
                    nc.vector.reciprocal_approx_fast(rec[:], ud[:])
                    u = nrm.tile([D, 512], F32, tag="u")
                    nc.vector.tensor_copy(u[:], pvt[0:D, :])
                    bc = nrm.tile([D, 512], F32, tag="bc")
                    nc.gpsimd.partition_broadcast(bc[:], rec[:])
                    if h_par == 0:
                        nc.vector.tensor_mul(
                            o_t[a][qt][0:D, :], u[:], bc[:]
                        )
                    else:
                        tmp = nrm.tile([D, 512], BF16, tag="tmp")
                        nc.vector.tensor_mul(tmp[:], u[:], bc[:])
                        # shift to partitions 64..127 (DVE cannot)
                        nc.sync.dma_start(o_t[a][qt][D:2 * D, :], tmp[:])

            # ---- schedule ---------------------------------------------
            Q = lambda jt, dst, st: (lambda: emit_q(jt, dst, st))
            V = lambda st: (lambda: emit_v(st))
            Pp = lambda st, ep, fine=False: (
                lambda: emit_proj_pair(st, ep, fine_store=fine)
            )

            # Each stretch's fills lead with the NEXT stretch's QK groups so
            # their casts are done well before that stretch's first sc.
            # Closing phase uses the smallest (qt=0) stretches so the end of
            # the kernel is never exp-limited.
            emit_q(0, 0, 0)
            emit_q(0, 1, 0)
            emit_attn(0, 0, boot_vs=True,
                      fills=[Q(1, 0, 0), Q(1, 1, 0)])        # S1 + V0..3
            emit_attn(1, 0, fills=[Q(0, 0, 1), Q(0, 1, 1),
                                   V(4)])                    # S2
            emit_attn(0, 1, fills=[Q(1, 0, 1), Q(1, 1, 1),
                                   V(5), V(6), V(7)],
                      fill_caps=(None, None, 4, 5, 6))       # S3
            emit_attn(1, 1, fills=[Q(2, 0, 0), Q(2, 1, 0),
                                   V(8)])                    # S4
            emit_attn(2, 0, fills=[Q(2, 0, 1), Q(2, 1, 1)])  # S5
            emit_attn(2, 1, fills=[Q(3, 0, 0), Q(3, 1, 0),
                                   V(9), V(10)])             # S6
            emit_attn(3, 0, fills=[Q(3, 0, 1), Q(3, 1, 1)])  # S7
            emit_attn(3, 1, fills=[Q(0, 1, 2), Q(0, 1, 3),
                                   Q(0, 0, 3), V(11), V(12)])  # S8
            emit_attn(0, 3, fills=[Q(1, 1, 2), Q(1, 1, 3),
                                   Q(1, 0, 3), V(13), V(14),
                                   V(15)],
                      fill_caps=(None, None, None,
                                 12, 13, 14))                # S9
            emit_attn(1, 3, fills=[Q(2, 1, 2), Q(2, 1, 3),
                                   Q(2, 0, 3), Pp(0, 0)])    # S10
            emit_attn(2, 3, fills=[Q(3, 1, 2), Q(3, 1, 3),
                                   Q(3, 0, 3), Pp(0, 1)])    # S11
            emit_attn(3, 3, fills=[Q(0, 0, 2), Pp(0, 2),
                                   Pp(0, 3), Pp(1, 0)])      # S12
            emit_attn(0, 2, fills=[Q(1, 0, 2), Pp(1, 1),
                                   Pp(1, 2)])                # S13
            emit_attn(1, 2, fills=[Q(2, 0, 2), Pp(1, 3),
                                   Pp(3, 0)])                # S14
            emit_attn(2, 2, fills=[Q(3, 0, 2)])              # S15
            emit_attn(3, 2, fills=[Pp(3, 1), Pp(3, 2),
                                   Pp(3, 3)],
                      tail=True)                             # S16
            # tail: final projections for st=2
            emit_proj_pair(2, 0)
            emit_proj_pair(2, 1)
            emit_proj_pair(2, 2)
            emit_proj_pair(2, 3, fine_store=True)

    nc.compile()
    return nc


def _causal_mask128() -> np.ndarray:
    p = np.arange(P)[:, None]
    c = np.arange(P)[None, :]
    return (c >= p).astype(np.float32).astype(NP_BF16)


def kernel(input, attention_mask, Wq, Wk, Wv, Wp, _profile=False):
    input = np.asarray(input, dtype=np.float32)
    attention_mask = np.asarray(attention_mask)
    Wq, Wk, Wv, Wp = (np.asarray(w, dtype=np.float32) for w in (Wq, Wk, Wv, Wp))

    mask_all = bool(attention_mask.all())
    nc = build_program(apply_key_mask=not mask_all)

    cm = _causal_mask128()
    in_maps = []
    for core in range(8):
        b, g = core // 2, core % 2
        rows = slice(g * JC, (g + 1) * JC)
        m = {
            "xT": np.ascontiguousarray(input[b].T).astype(NP_BF16),
            "wqT": np.ascontiguousarray(Wq[rows].T).astype(NP_BF16),
            "wkT": np.ascontiguousarray(Wk[rows].T).astype(NP_BF16),
            "wvT": np.ascontiguousarray(Wv[rows].T).astype(NP_BF16),
            "wpT": np.ascontiguousarray(Wp[:, rows].T).astype(NP_BF16),
            "cmask": cm,
            "ident": np.eye(D, dtype=np.float32).astype(NP_BF16),
        }
        if not mask_all:
            km = attention_mask[b].astype(np.float32)  # [S]
            m["kmaskT"] = np.ascontiguousarray(km.reshape(NKT, P).T)
        in_maps.append(m)

    res = run_bass_kernel_spmd(
        nc, in_maps, core_ids=list(range(8)), trace=_profile
    )

    out = np.empty((B, S, E), dtype=np.float32)
    for b in range(B):
        acc = (res.results[2 * b]["outp"].astype(np.float32)
               + res.results[2 * b + 1]["outp"].astype(np.float32))
        out[b] = acc.T
    if _profile:
        return out, res
    return out
